# revision 30
# baseline (speedup 1.0000x reference)
"""Trainium2 Bass kernel for the DecoderCRF loss (B=64, S=512, D=512, T=12).

Math
----
reference loss = sum_b [ logZ_b - gold_b ] with feats = x @ W.T + b.

For the transitions matrix this problem ships (row START == -1e4, col
STOP == -1e4, everything else 0) and an all-ones mask, the forward
recursion collapses exactly (verified vs a float64 port of the reference):

    logZ_b  = sum_t log( sum_{j=0..9} exp(feats[b,t,j]) )
    gold_b  = sum_t feats[b,t,tags[b,t]]

Layout strategy (final; 38.1us baseline -> ~21.4-22.1us measured)
------------------------------------------------------------
v2 was tensor-engine bound (59 matmuls = 27us busy incl the HAM cold
clock, plus 6us ScalarE exp + 5us DVE on 10-partition tiles).  Now:

* x ships as fp8(e4m3), W pre-scaled by 32 in fp8 -> DMA halves to
  2MB/core.  Host-simulated pipeline rel err: 3.6e-05 (gate 2e-2).
* Plain fp8 matmuls (DoubleRow streams 2N rhs entries - no cycle win -
  and rejects tile_position packing in walrus): 8 halves x 4 d-chunks
  of [128, 512].  PE col-tiling (tile_position=(0,32g)) packs up to 4
  halves' [10,512] outputs into one PSUM bank at offsets 0/32/64/96.
* DMA routing by measured queue behavior: the SWDGE (gpsimd) path
  streams ~300GB/s but its first transfer lands ~8us after emission,
  so it carries h0-h5; the two HWDGE queues (sync/scalar - start
  immediately, ~100GB/s each) carry h6/h7, and the PE consumes THOSE
  first so it never waits for the SWDGE pipe to wake.  (Giving HWDGE
  more than 2 slabs crowds the shared SDMA engines and delays the
  SWDGE wake - measured net loss.)
* A burst of warmup matmuls on memset-zero tiles runs while the x DMA
  streams, burning the HAM cold-clock window (PE ramps 1.2->2.4GHz
  after ~3.4us of sustained activity) so real matmuls run warm
  (215ns issue cadence per [128,512] when the package isn't power-
  throttled by the other 7 cores running the same kernel).
* Evacuation: banks A(h6,h7,h0,h1)/B(h2-h4) each get one wide DVE
  tensor_copy [128,512] f32->bf16 + one 128KB sync-queue DMA out,
  both completing during later matmuls; the last-consumed h5 sits in
  its own PSUM bank so only a [10,512] cast + 10KB out trail the
  final matmul.  Junk partitions between the 10-row groups ship and
  are ignored on host.
* No on-device exp/reductions: feats ship out and the O(B*S*T) finish
  (exp/log/sum/gather in f64) runs on host, like v2's log/bias finish.
* Remaining time is mostly fixed cost: ~4us NEFF startup (engine
  program loads + barriers), ~2.4us teardown, ~8us SWDGE pipe latency
  (warmup-overlapped), stream ~7us, tail ~2.5us.

Non-conforming inputs (different transitions pattern / mask / tag range)
fall back to a faithful numpy port of the reference.
"""

from contextlib import ExitStack

import numpy as np

N_CORES = 8
B, S, D = 64, 512, 512
T = 12
NT = 10          # tags that can actually appear / participate in the LSE
START, STOP = 10, 11
NEG = -10000.0
BS = B // N_CORES          # batch elements per core
R = BS * S                 # s-rows per core (4096)
N_HALF = 8                 # 512-col halves per core (one batch element each)
HALF = R // N_HALF         # 512
N_SLAB = 8                 # x DMA slabs per core (one half each)
N_WARM = 7                 # HAM warmup matmuls issued before real data lands
WSCALE = 32.0              # W is shipped as 32*W in fp8; host divides out

_NC_CACHE = None


def _build_nc():
    import concourse.bacc as bacc
    import concourse.mybir as mybir
    import concourse.tile as tile

    f32 = mybir.dt.float32
    bf16 = mybir.dt.bfloat16
    f8 = mybir.dt.float8e4
    nc = bacc.Bacc("TRN2", target_bir_lowering=False, num_swdge_queues=1)

    # slab k holds half k: [partition p, dc, s] with d = dc*128 + p,
    # global row = 512*k + s.  Per-partition data is one contiguous 2KB run.
    xt_d = nc.dram_tensor("xt", [N_SLAB, 128, 4, HALF], f8, kind="ExternalInput")
    wt_d = nc.dram_tensor("wt", [128, 4, NT], f8, kind="ExternalInput")
    out_d = nc.dram_tensor("out_e", [2, 128, HALF], bf16, kind="ExternalOutput")
    outc_d = nc.dram_tensor("out_c", [NT, HALF], bf16, kind="ExternalOutput")

    with tile.TileContext(nc) as tc, ExitStack() as ctx:
        consts = ctx.enter_context(tc.tile_pool(name="consts", bufs=1))
        xp = ctx.enter_context(tc.tile_pool(name="xp", bufs=N_SLAB))
        ep = ctx.enter_context(tc.tile_pool(name="ep", bufs=2))
        pw = ctx.enter_context(tc.tile_pool(name="pw", bufs=1, space="PSUM"))
        pp = ctx.enter_context(tc.tile_pool(name="pp", bufs=2, space="PSUM"))

        # tiny SWDGE kick: absorbs any one-time SWDGE/SDMA startup latency
        kick_sb = consts.tile([1, 64], f8, tag="kick")
        nc.gpsimd.dma_start(out=kick_sb, in_=xt_d[0, 0, 0, 0:64])

        # HWDGE (starts immediately, ~100GB/s/queue) carries h6 then h5 on
        # sync and h7 on scalar; the PE consumes those three first, while
        # the SWDGE pipe (fast but ~8us wake) streams h0-h4 behind them.
        # (2 slabs per HWDGE queue measured correct; 4 per queue races.)
        xt_tiles = [None] * N_SLAB
        for k in (6, 7, 5, 0, 1, 2, 3, 4):
            xt_sb = xp.tile([128, 4, HALF], f8, tag="xt")
            if k in (6, 5):
                nc.sync.dma_start(out=xt_sb, in_=xt_d[k])
            elif k == 7:
                nc.scalar.dma_start(out=xt_sb, in_=xt_d[k])
            else:
                nc.gpsimd.dma_start(out=xt_sb, in_=xt_d[k])
            xt_tiles[k] = xt_sb

        wt_sb = consts.tile([128, 4, NT], f8)
        nc.sync.dma_start(out=wt_sb, in_=wt_d[:, :, :])

        # HAM warmup: zero matmuls with no DMA dependency keep the PE busy
        # through its ~3.4us cold-clock window while x streams in.
        wz = consts.tile([128, NT], f8, tag="wz")
        nc.vector.memset(wz, 0.0)
        xz = consts.tile([128, HALF], f8, tag="xz")
        nc.vector.memset(xz, 0.0)
        ps_w = pw.tile([NT, HALF], f32, tag="psw")
        for _ in range(N_WARM):
            nc.tensor.matmul(ps_w, lhsT=wz, rhs=xz, start=True, stop=True)


        # PE consumes the HWDGE halves h6,h7,h5 FIRST, then chases the
        # SWDGE stream h0..h4.  Three mid-warmup zero matmuls after h5
        # keep the PE clock hot across the residual SWDGE-wake gap.  The
        # last-consumed h4 sits alone in bank C so only a [10,512] cast +
        # 10KB out trail the final matmul.
        BANK_HALVES = ((6, 7, 5, 0), (1, 2, 3))
        for bank, halves in enumerate(BANK_HALVES):
            ps = pp.tile([128, HALF], f32, tag="ps")
            for g, h in enumerate(halves):
                for dc in range(4):   # four 128-deep d-chunks
                    nc.tensor.matmul(
                        ps[32 * g : 32 * g + NT, :],
                        lhsT=wt_sb[:, dc],
                        rhs=xt_tiles[h][:, dc],
                        start=(dc == 0),
                        stop=(dc == 3),
                        tile_position=(0, 32 * g),
                    )
                if bank == 0 and h == 5:
                    for _ in range(3):   # mid-warmups bridge the wake gap
                        nc.tensor.matmul(ps_w, lhsT=wz, rhs=xz,
                                         start=True, stop=True)
            e_sb = ep.tile([128, HALF], bf16, tag="e")
            nc.vector.tensor_copy(out=e_sb, in_=ps)
            nc.sync.dma_start(out=out_d[bank], in_=e_sb)

        ps_c = pp.tile([NT, HALF], f32, tag="psc")
        for dc in range(4):
            nc.tensor.matmul(
                ps_c,
                lhsT=wt_sb[:, dc],
                rhs=xt_tiles[4][:, dc],
                start=(dc == 0),
                stop=(dc == 3),
            )
        ec_sb = ep.tile([NT, HALF], bf16, tag="ec")
        nc.vector.tensor_copy(out=ec_sb, in_=ps_c)
        nc.sync.dma_start(out=outc_d[:, :], in_=ec_sb)

    nc.compile()
    return nc


def _get_nc():
    global _NC_CACHE
    if _NC_CACHE is None:
        _NC_CACHE = _build_nc()
    return _NC_CACHE


def _fast_path_ok(transitions, tags, mask):
    if transitions.shape != (T, T) or tags.min() < 0 or tags.max() >= NT:
        return False
    if not np.all(mask == 1):
        return False
    t2 = np.asarray(transitions, np.float64).copy()
    if not (np.all(t2[START, :] == NEG) and np.all(t2[:, STOP] == NEG)):
        return False
    t2[START, :] = 0.0
    t2[:, STOP] = 0.0
    return bool(np.all(t2 == 0.0))


def _reference_numpy(input_var, W, b, transitions, tags, mask):
    """Faithful float64 port of the reference (fallback only)."""
    x = np.asarray(input_var, np.float64)
    Wf = np.asarray(W, np.float64)
    bf = np.asarray(b, np.float64)
    tr = np.asarray(transitions, np.float64)
    mf = np.asarray(mask, np.float64)
    Bn, Sn, Dn = x.shape
    feats = (x.reshape(-1, Dn) @ Wf.T + bf).reshape(Bn, Sn, -1)
    fv = np.full((Bn, T), NEG)
    fv[:, START] = 0.0
    for t in range(Sn):
        tv = fv[:, None, :] + tr[None] + feats[:, t][:, :, None]
        m = tv.max(axis=2)
        new = m + np.log(np.exp(tv - m[:, :, None]).sum(axis=2))
        fv = new * mf[:, t : t + 1] + fv * (1 - mf[:, t : t + 1])
    fin = fv + tr[STOP][None]
    mm = fin.max(axis=1)
    alpha = mm + np.log(np.exp(fin - mm[:, None]).sum(axis=1))
    score0 = tr[tags[:, 0], START]
    emit = np.take_along_axis(feats[:, :-1], tags[:, :-1, None], axis=2)[..., 0]
    emit_sum = (emit * mf[:, :-1]).sum(axis=1)
    trs = tr[tags[:, 1:], tags[:, :-1]]
    trans_sum = (trs * mf[:, 1:]).sum(axis=1)
    last_idx = np.asarray(mask).sum(axis=1).astype(np.int64) - 1
    last_tags = np.take_along_axis(tags, last_idx[:, None], axis=1)[:, 0]
    last_emit = np.take_along_axis(feats[:, -1], last_tags[:, None], axis=1)[:, 0]
    gold = score0 + emit_sum + trans_sum + tr[STOP, last_tags] + last_emit * mf[:, -1]
    return np.float32((alpha - gold).sum())


def _make_in_maps(input_var, W, b, tags):
    import ml_dtypes

    f8 = ml_dtypes.float8_e4m3
    # wt[p, dc, j] = 32*W[j, dc*128 + p]
    w32 = WSCALE * np.asarray(W[:NT], np.float32)
    wt = np.ascontiguousarray(
        w32.T.reshape(4, 128, NT).transpose(1, 0, 2)
    ).astype(f8)

    x8 = input_var.reshape(B * S, D).astype(f8)   # one big cast
    in_maps = []
    for c in range(N_CORES):
        xc = x8[c * R : (c + 1) * R]              # [4096, 512]
        # xt[k, p, dc, s] = x[k*512+s, dc*128 + p]
        xt = np.ascontiguousarray(
            xc.T.reshape(4, 128, N_SLAB, HALF).transpose(2, 1, 0, 3)
        )
        in_maps.append({"xt": xt, "wt": wt})
    return in_maps


def kernel(input_var, W, b, transitions, tags, mask):
    from concourse.bass_utils import run_bass_kernel_spmd

    input_var = np.asarray(input_var)
    W = np.asarray(W)
    b = np.asarray(b)
    transitions = np.asarray(transitions)
    tags = np.asarray(tags)
    mask = np.asarray(mask)

    if not _fast_path_ok(transitions, tags, mask):
        return _reference_numpy(input_var, W, b, transitions, tags, mask)

    nc = _get_nc()
    in_maps = _make_in_maps(input_var, W, b, tags)
    res = run_bass_kernel_spmd(nc, in_maps, list(range(N_CORES)))

    # out_e group (bank,g) holds half [6,7,5,0 | 1,2,3][...]; h4 is out_c
    Fb = np.stack([np.asarray(res.results[c]["out_e"]) for c in range(N_CORES)])
    Fb = Fb.reshape(N_CORES, 2, 4, 32, HALF)[:, :, :, :NT]   # drop junk rows
    Fc = np.stack([np.asarray(res.results[c]["out_c"]) for c in range(N_CORES)])
    Fg = Fb.reshape(N_CORES, 8, NT, HALF)
    F = np.empty_like(Fg)
    for slot, h in enumerate((6, 7, 5, 0, 1, 2, 3)):
        F[:, h] = Fg[:, slot]
    F[:, 4] = Fc
    f = F.astype(np.float64) / WSCALE + np.asarray(b, np.float64)[:NT][None, None, :, None]
    f = f.reshape(B, NT, S)                        # [b, j, t]
    m = f.max(axis=1)
    lse = m + np.log(np.exp(f - m[:, None, :]).sum(axis=1))   # [B, S]
    gold = np.take_along_axis(f, tags[:, None, :].astype(np.int64), axis=1)[:, 0]
    return np.float32((lse - gold).sum())


# revision 31
# speedup vs baseline: 1.0831x; 1.0831x over previous
"""Trainium2 Bass kernel for the DecoderCRF loss (B=64, S=512, D=512, T=12).

Math
----
reference loss = sum_b [ logZ_b - gold_b ] with feats = x @ W.T + b.

For the transitions matrix this problem ships (row START == -1e4, col
STOP == -1e4, everything else 0) and an all-ones mask, the forward
recursion collapses exactly (verified vs a float64 port of the reference):

    logZ_b  = sum_t log( sum_{j=0..9} exp(feats[b,t,j]) )
    gold_b  = sum_t feats[b,t,tags[b,t]]

Layout strategy (final; 38.1us baseline -> ~21.4-22.1us measured)
------------------------------------------------------------
v2 was tensor-engine bound (59 matmuls = 27us busy incl the HAM cold
clock, plus 6us ScalarE exp + 5us DVE on 10-partition tiles).  Now:

* x ships as fp8(e4m3), W pre-scaled by 32 in fp8 -> DMA halves to
  2MB/core.  Host-simulated pipeline rel err: 3.6e-05 (gate 2e-2).
* Plain fp8 matmuls (DoubleRow streams 2N rhs entries - no cycle win -
  and rejects tile_position packing in walrus): 8 halves x 4 d-chunks
  of [128, 512].  PE col-tiling (tile_position=(0,32g)) packs up to 4
  halves' [10,512] outputs into one PSUM bank at offsets 0/32/64/96.
* DMA routing by measured queue behavior: the SWDGE (gpsimd) path
  streams ~300GB/s but its first transfer lands ~8us after emission,
  so it carries h0-h5; the two HWDGE queues (sync/scalar - start
  immediately, ~100GB/s each) carry h6/h7, and the PE consumes THOSE
  first so it never waits for the SWDGE pipe to wake.  (Giving HWDGE
  more than 2 slabs crowds the shared SDMA engines and delays the
  SWDGE wake - measured net loss.)
* A burst of warmup matmuls on memset-zero tiles runs while the x DMA
  streams, burning the HAM cold-clock window (PE ramps 1.2->2.4GHz
  after ~3.4us of sustained activity) so real matmuls run warm
  (215ns issue cadence per [128,512] when the package isn't power-
  throttled by the other 7 cores running the same kernel).
* Evacuation: banks A(h6,h7,h0,h1)/B(h2-h4) each get one wide DVE
  tensor_copy [128,512] f32->bf16 + one 128KB sync-queue DMA out,
  both completing during later matmuls; the last-consumed h5 sits in
  its own PSUM bank so only a [10,512] cast + 10KB out trail the
  final matmul.  Junk partitions between the 10-row groups ship and
  are ignored on host.
* No on-device exp/reductions: feats ship out and the O(B*S*T) finish
  (exp/log/sum/gather in f64) runs on host, like v2's log/bias finish.
* Remaining time is mostly fixed cost: ~4us NEFF startup (engine
  program loads + barriers), ~2.4us teardown, ~8us SWDGE pipe latency
  (warmup-overlapped), stream ~7us, tail ~2.5us.

Non-conforming inputs (different transitions pattern / mask / tag range)
fall back to a faithful numpy port of the reference.
"""

from contextlib import ExitStack

import numpy as np

N_CORES = 8
B, S, D = 64, 512, 512
T = 12
NT = 10          # tags that can actually appear / participate in the LSE
START, STOP = 10, 11
NEG = -10000.0
BS = B // N_CORES          # batch elements per core
R = BS * S                 # s-rows per core (4096)
N_HALF = 8                 # 512-col halves per core (one batch element each)
HALF = R // N_HALF         # 512
N_SLAB = 8                 # x DMA slabs per core (one half each)
N_WARM = 7                 # HAM warmup matmuls issued before real data lands
WSCALE = 32.0              # W is shipped as 32*W in fp8; host divides out

_NC_CACHE = None


def _build_nc():
    import concourse.bacc as bacc
    import concourse.mybir as mybir
    import concourse.tile as tile

    f32 = mybir.dt.float32
    bf16 = mybir.dt.bfloat16
    f8 = mybir.dt.float8e4
    nc = bacc.Bacc("TRN2", target_bir_lowering=False, num_swdge_queues=1)

    # slab k holds half k: [partition p, dc, s] with d = dc*128 + p,
    # global row = 512*k + s.  Per-partition data is one contiguous 2KB run.
    xt_d = nc.dram_tensor("xt", [N_SLAB, 128, 4, HALF], f8, kind="ExternalInput")
    wt_d = nc.dram_tensor("wt", [128, 4, NT], f8, kind="ExternalInput")
    out_d = nc.dram_tensor("out_e", [2, 128, HALF], bf16, kind="ExternalOutput")
    outc_d = nc.dram_tensor("out_c", [NT, HALF], bf16, kind="ExternalOutput")

    with tile.TileContext(nc) as tc, ExitStack() as ctx:
        consts = ctx.enter_context(tc.tile_pool(name="consts", bufs=1))
        xp = ctx.enter_context(tc.tile_pool(name="xp", bufs=N_SLAB))
        ep = ctx.enter_context(tc.tile_pool(name="ep", bufs=2))
        pw = ctx.enter_context(tc.tile_pool(name="pw", bufs=1, space="PSUM"))
        pp = ctx.enter_context(tc.tile_pool(name="pp", bufs=2, space="PSUM"))

        # tiny SWDGE kick: absorbs any one-time SWDGE/SDMA startup latency
        kick_sb = consts.tile([1, 64], f8, tag="kick")
        nc.gpsimd.dma_start(out=kick_sb, in_=xt_d[0, 0, 0, 0:64])

        # h0-h5 stream on the fast SWDGE path in consumption order; the two
        # HWDGE queues (slower but independent) each carry one of the LAST-
        # emitted halves, which the PE consumes FIRST (they arrive ~7us
        # while the SWDGE pipe wakes at ~11.5us).  A third HWDGE slab
        # delays the SWDGE wake - measured net loss.
        xt_tiles = []
        for k in range(N_SLAB):
            xt_sb = xp.tile([128, 4, HALF], f8, tag="xt")
            if k == 6:
                nc.sync.dma_start(out=xt_sb, in_=xt_d[k])
            elif k == 7:
                nc.scalar.dma_start(out=xt_sb, in_=xt_d[k])
            else:
                nc.gpsimd.dma_start(out=xt_sb, in_=xt_d[k])
            xt_tiles.append(xt_sb)

        wt_sb = consts.tile([128, 4, NT], f8)
        nc.sync.dma_start(out=wt_sb, in_=wt_d[:, :, :])

        # HAM warmup: zero matmuls with no DMA dependency keep the PE busy
        # through its ~3.4us cold-clock window while x streams in.
        wz = consts.tile([128, NT], f8, tag="wz")
        nc.vector.memset(wz, 0.0)
        xz = consts.tile([128, HALF], f8, tag="xz")
        nc.vector.memset(xz, 0.0)
        ps_w = pw.tile([NT, HALF], f32, tag="psw")
        for _ in range(N_WARM):
            nc.tensor.matmul(ps_w, lhsT=wz, rhs=xz, start=True, stop=True)


        # PE consumes h6/h7 FIRST, then chases the SWDGE stream h0..h5.
        # The last-consumed h5 sits alone in bank C so only a [10,512]
        # cast + 10KB out trail the final matmul.
        BANK_HALVES = ((6, 7, 0, 1), (2, 3, 4))
        for bank, halves in enumerate(BANK_HALVES):
            ps = pp.tile([128, HALF], f32, tag="ps")
            for g, h in enumerate(halves):
                for dc in range(4):   # four 128-deep d-chunks
                    nc.tensor.matmul(
                        ps[32 * g : 32 * g + NT, :],
                        lhsT=wt_sb[:, dc],
                        rhs=xt_tiles[h][:, dc],
                        start=(dc == 0),
                        stop=(dc == 3),
                        tile_position=(0, 32 * g),
                    )
            e_sb = ep.tile([128, HALF], bf16, tag="e")
            nc.vector.tensor_copy(out=e_sb, in_=ps)
            nc.sync.dma_start(out=out_d[bank], in_=e_sb)

        ps_c = pp.tile([NT, HALF], f32, tag="psc")
        for dc in range(4):
            nc.tensor.matmul(
                ps_c,
                lhsT=wt_sb[:, dc],
                rhs=xt_tiles[5][:, dc],
                start=(dc == 0),
                stop=(dc == 3),
            )
        ec_sb = ep.tile([NT, HALF], bf16, tag="ec")
        nc.vector.tensor_copy(out=ec_sb, in_=ps_c)
        nc.sync.dma_start(out=outc_d[:, :], in_=ec_sb)

    nc.compile()
    return nc


def _get_nc():
    global _NC_CACHE
    if _NC_CACHE is None:
        _NC_CACHE = _build_nc()
    return _NC_CACHE


def _fast_path_ok(transitions, tags, mask):
    if transitions.shape != (T, T) or tags.min() < 0 or tags.max() >= NT:
        return False
    if not np.all(mask == 1):
        return False
    t2 = np.asarray(transitions, np.float64).copy()
    if not (np.all(t2[START, :] == NEG) and np.all(t2[:, STOP] == NEG)):
        return False
    t2[START, :] = 0.0
    t2[:, STOP] = 0.0
    return bool(np.all(t2 == 0.0))


def _reference_numpy(input_var, W, b, transitions, tags, mask):
    """Faithful float64 port of the reference (fallback only)."""
    x = np.asarray(input_var, np.float64)
    Wf = np.asarray(W, np.float64)
    bf = np.asarray(b, np.float64)
    tr = np.asarray(transitions, np.float64)
    mf = np.asarray(mask, np.float64)
    Bn, Sn, Dn = x.shape
    feats = (x.reshape(-1, Dn) @ Wf.T + bf).reshape(Bn, Sn, -1)
    fv = np.full((Bn, T), NEG)
    fv[:, START] = 0.0
    for t in range(Sn):
        tv = fv[:, None, :] + tr[None] + feats[:, t][:, :, None]
        m = tv.max(axis=2)
        new = m + np.log(np.exp(tv - m[:, :, None]).sum(axis=2))
        fv = new * mf[:, t : t + 1] + fv * (1 - mf[:, t : t + 1])
    fin = fv + tr[STOP][None]
    mm = fin.max(axis=1)
    alpha = mm + np.log(np.exp(fin - mm[:, None]).sum(axis=1))
    score0 = tr[tags[:, 0], START]
    emit = np.take_along_axis(feats[:, :-1], tags[:, :-1, None], axis=2)[..., 0]
    emit_sum = (emit * mf[:, :-1]).sum(axis=1)
    trs = tr[tags[:, 1:], tags[:, :-1]]
    trans_sum = (trs * mf[:, 1:]).sum(axis=1)
    last_idx = np.asarray(mask).sum(axis=1).astype(np.int64) - 1
    last_tags = np.take_along_axis(tags, last_idx[:, None], axis=1)[:, 0]
    last_emit = np.take_along_axis(feats[:, -1], last_tags[:, None], axis=1)[:, 0]
    gold = score0 + emit_sum + trans_sum + tr[STOP, last_tags] + last_emit * mf[:, -1]
    return np.float32((alpha - gold).sum())


def _make_in_maps(input_var, W, b, tags):
    import ml_dtypes

    f8 = ml_dtypes.float8_e4m3
    # wt[p, dc, j] = 32*W[j, dc*128 + p]
    w32 = WSCALE * np.asarray(W[:NT], np.float32)
    wt = np.ascontiguousarray(
        w32.T.reshape(4, 128, NT).transpose(1, 0, 2)
    ).astype(f8)

    x8 = input_var.reshape(B * S, D).astype(f8)   # one big cast
    in_maps = []
    for c in range(N_CORES):
        xc = x8[c * R : (c + 1) * R]              # [4096, 512]
        # xt[k, p, dc, s] = x[k*512+s, dc*128 + p]
        xt = np.ascontiguousarray(
            xc.T.reshape(4, 128, N_SLAB, HALF).transpose(2, 1, 0, 3)
        )
        in_maps.append({"xt": xt, "wt": wt})
    return in_maps


def kernel(input_var, W, b, transitions, tags, mask):
    from concourse.bass_utils import run_bass_kernel_spmd

    input_var = np.asarray(input_var)
    W = np.asarray(W)
    b = np.asarray(b)
    transitions = np.asarray(transitions)
    tags = np.asarray(tags)
    mask = np.asarray(mask)

    if not _fast_path_ok(transitions, tags, mask):
        return _reference_numpy(input_var, W, b, transitions, tags, mask)

    nc = _get_nc()
    in_maps = _make_in_maps(input_var, W, b, tags)
    res = run_bass_kernel_spmd(nc, in_maps, list(range(N_CORES)))

    # out_e group (bank,g) holds half [6,7,0,1 | 2,3,4][...]; h5 is out_c
    Fb = np.stack([np.asarray(res.results[c]["out_e"]) for c in range(N_CORES)])
    Fb = Fb.reshape(N_CORES, 2, 4, 32, HALF)[:, :, :, :NT]   # drop junk rows
    Fc = np.stack([np.asarray(res.results[c]["out_c"]) for c in range(N_CORES)])
    Fg = Fb.reshape(N_CORES, 8, NT, HALF)
    F = np.empty_like(Fg)
    for slot, h in enumerate((6, 7, 0, 1, 2, 3, 4)):
        F[:, h] = Fg[:, slot]
    F[:, 5] = Fc
    f = F.astype(np.float64) / WSCALE + np.asarray(b, np.float64)[:NT][None, None, :, None]
    f = f.reshape(B, NT, S)                        # [b, j, t]
    m = f.max(axis=1)
    lse = m + np.log(np.exp(f - m[:, None, :]).sum(axis=1))   # [B, S]
    gold = np.take_along_axis(f, tags[:, None, :].astype(np.int64), axis=1)[:, 0]
    return np.float32((lse - gold).sum())


# revision 32
# speedup vs baseline: 1.1263x; 1.0399x over previous
"""Trainium2 Bass kernel for the DecoderCRF loss (B=64, S=512, D=512, T=12).

Math
----
reference loss = sum_b [ logZ_b - gold_b ] with feats = x @ W.T + b.

For the transitions matrix this problem ships (row START == -1e4, col
STOP == -1e4, everything else 0) and an all-ones mask, the forward
recursion collapses exactly (verified vs a float64 port of the reference):

    logZ_b  = sum_t log( sum_{j=0..9} exp(feats[b,t,j]) )
    gold_b  = sum_t feats[b,t,tags[b,t]]

Layout strategy (final; 38.1us baseline -> ~21.4-22.1us measured)
------------------------------------------------------------
v2 was tensor-engine bound (59 matmuls = 27us busy incl the HAM cold
clock, plus 6us ScalarE exp + 5us DVE on 10-partition tiles).  Now:

* x ships as fp8(e4m3), W pre-scaled by 32 in fp8 -> DMA halves to
  2MB/core.  Host-simulated pipeline rel err: 3.6e-05 (gate 2e-2).
* Plain fp8 matmuls (DoubleRow streams 2N rhs entries - no cycle win -
  and rejects tile_position packing in walrus): 8 halves x 4 d-chunks
  of [128, 512].  PE col-tiling (tile_position=(0,32g)) packs up to 4
  halves' [10,512] outputs into one PSUM bank at offsets 0/32/64/96.
* DMA routing by measured queue behavior: the SWDGE (gpsimd) path
  streams ~300GB/s but its first transfer lands ~8us after emission,
  so it carries h0-h5; the two HWDGE queues (sync/scalar - start
  immediately, ~100GB/s each) carry h6/h7, and the PE consumes THOSE
  first so it never waits for the SWDGE pipe to wake.  (Giving HWDGE
  more than 2 slabs crowds the shared SDMA engines and delays the
  SWDGE wake - measured net loss.)
* A burst of warmup matmuls on memset-zero tiles runs while the x DMA
  streams, burning the HAM cold-clock window (PE ramps 1.2->2.4GHz
  after ~3.4us of sustained activity) so real matmuls run warm
  (215ns issue cadence per [128,512] when the package isn't power-
  throttled by the other 7 cores running the same kernel).
* Evacuation: banks A(h6,h7,h0,h1)/B(h2-h4) each get one wide DVE
  tensor_copy [128,512] f32->bf16 + one 128KB sync-queue DMA out,
  both completing during later matmuls; the last-consumed h5 sits in
  its own PSUM bank so only a [10,512] cast + 10KB out trail the
  final matmul.  Junk partitions between the 10-row groups ship and
  are ignored on host.
* No on-device exp/reductions: feats ship out and the O(B*S*T) finish
  (exp/log/sum/gather in f64) runs on host, like v2's log/bias finish.
* Remaining time is mostly fixed cost: ~4us NEFF startup (engine
  program loads + barriers), ~2.4us teardown, ~8us SWDGE pipe latency
  (warmup-overlapped), stream ~7us, tail ~2.5us.

Non-conforming inputs (different transitions pattern / mask / tag range)
fall back to a faithful numpy port of the reference.
"""

from contextlib import ExitStack

import numpy as np

N_CORES = 8
B, S, D = 64, 512, 512
T = 12
NT = 10          # tags that can actually appear / participate in the LSE
START, STOP = 10, 11
NEG = -10000.0
BS = B // N_CORES          # batch elements per core
R = BS * S                 # s-rows per core (4096)
N_HALF = 8                 # 512-col halves per core (one batch element each)
HALF = R // N_HALF         # 512
N_SLAB = 8                 # x DMA slabs per core (one half each)
N_WARM = 7                 # HAM warmup matmuls issued before real data lands
WSCALE = 32.0              # W is shipped as 32*W in fp8; host divides out

_NC_CACHE = None


def _build_nc():
    import concourse.bacc as bacc
    import concourse.mybir as mybir
    import concourse.tile as tile

    f32 = mybir.dt.float32
    bf16 = mybir.dt.bfloat16
    f8 = mybir.dt.float8e4
    nc = bacc.Bacc("TRN2", target_bir_lowering=False, num_swdge_queues=1)

    # slab k holds half k: [partition p, dc, s] with d = dc*128 + p,
    # global row = 512*k + s.  Per-partition data is one contiguous 2KB run.
    xt_d = nc.dram_tensor("xt", [N_SLAB, 128, 4, HALF], f8, kind="ExternalInput")
    wt_d = nc.dram_tensor("wt", [128, 4, NT], f8, kind="ExternalInput")
    out_d = nc.dram_tensor("out_e", [2, 128, HALF], bf16, kind="ExternalOutput")
    outc_d = nc.dram_tensor("out_c", [NT, HALF], bf16, kind="ExternalOutput")

    with tile.TileContext(nc) as tc, ExitStack() as ctx:
        consts = ctx.enter_context(tc.tile_pool(name="consts", bufs=1))
        xp = ctx.enter_context(tc.tile_pool(name="xp", bufs=N_SLAB))
        ep = ctx.enter_context(tc.tile_pool(name="ep", bufs=2))
        pw = ctx.enter_context(tc.tile_pool(name="pw", bufs=1, space="PSUM"))
        pp = ctx.enter_context(tc.tile_pool(name="pp", bufs=2, space="PSUM"))

        # tiny SWDGE kick: absorbs any one-time SWDGE/SDMA startup latency
        kick_sb = consts.tile([1, 64], f8, tag="kick")
        nc.gpsimd.dma_start(out=kick_sb, in_=xt_d[0, 0, 0, 0:64])

        # h0-h5 stream on the fast SWDGE path in consumption order; the two
        # HWDGE queues (slower but independent) each carry one of the LAST-
        # emitted halves, which the PE consumes FIRST (they arrive ~7us
        # while the SWDGE pipe wakes at ~11.5us).  A third HWDGE slab
        # delays the SWDGE wake - measured net loss.
        xt_tiles = []
        for k in range(N_SLAB):
            xt_sb = xp.tile([128, 4, HALF], f8, tag="xt")
            if k == 6:
                nc.sync.dma_start(out=xt_sb, in_=xt_d[k])
            elif k == 7:
                nc.scalar.dma_start(out=xt_sb, in_=xt_d[k])
            else:
                nc.gpsimd.dma_start(out=xt_sb, in_=xt_d[k])
            xt_tiles.append(xt_sb)

        wt_sb = consts.tile([128, 4, NT], f8)
        nc.sync.dma_start(out=wt_sb, in_=wt_d[:, :, :])

        # HAM warmup: zero matmuls with no DMA dependency keep the PE busy
        # through its ~3.4us cold-clock window while x streams in.
        wz = consts.tile([128, NT], f8, tag="wz")
        nc.vector.memset(wz, 0.0)
        xz = consts.tile([128, HALF], f8, tag="xz")
        nc.vector.memset(xz, 0.0)
        ps_w = pw.tile([NT, HALF], f32, tag="psw")
        for _ in range(N_WARM):
            nc.tensor.matmul(ps_w, lhsT=wz, rhs=xz, start=True, stop=True)


        # PE consumes h6/h7 FIRST, then chases the SWDGE stream h0..h5.
        # The last-consumed h5 sits alone in bank C so only a [10,512]
        # cast + 10KB out trail the final matmul.
        BANK_HALVES = ((6, 7, 0, 1), (2, 3, 4))
        for bank, halves in enumerate(BANK_HALVES):
            ps = pp.tile([128, HALF], f32, tag="ps")
            for g, h in enumerate(halves):
                for dc in range(4):   # four 128-deep d-chunks
                    nc.tensor.matmul(
                        ps[32 * g : 32 * g + NT, :],
                        lhsT=wt_sb[:, dc],
                        rhs=xt_tiles[h][:, dc],
                        start=(dc == 0),
                        stop=(dc == 3),
                        tile_position=(0, 32 * g),
                    )
            e_sb = ep.tile([128, HALF], bf16, tag="e")
            nc.vector.tensor_copy(out=e_sb, in_=ps)
            nc.sync.dma_start(out=out_d[bank], in_=e_sb)

        ps_c = pp.tile([NT, HALF], f32, tag="psc")
        for dc in range(4):
            nc.tensor.matmul(
                ps_c,
                lhsT=wt_sb[:, dc],
                rhs=xt_tiles[5][:, dc],
                start=(dc == 0),
                stop=(dc == 3),
            )
        ec_sb = ep.tile([NT, HALF], bf16, tag="ec")
        nc.vector.tensor_copy(out=ec_sb, in_=ps_c)
        # scalar HWDGE queue is idle by now: the tail 10KB out emits in
        # parallel with bank B's 128KB out on sync instead of behind it
        nc.scalar.dma_start(out=outc_d[:, :], in_=ec_sb)

    nc.compile()
    return nc


def _get_nc():
    global _NC_CACHE
    if _NC_CACHE is None:
        _NC_CACHE = _build_nc()
    return _NC_CACHE


def _fast_path_ok(transitions, tags, mask):
    if transitions.shape != (T, T) or tags.min() < 0 or tags.max() >= NT:
        return False
    if not np.all(mask == 1):
        return False
    t2 = np.asarray(transitions, np.float64).copy()
    if not (np.all(t2[START, :] == NEG) and np.all(t2[:, STOP] == NEG)):
        return False
    t2[START, :] = 0.0
    t2[:, STOP] = 0.0
    return bool(np.all(t2 == 0.0))


def _reference_numpy(input_var, W, b, transitions, tags, mask):
    """Faithful float64 port of the reference (fallback only)."""
    x = np.asarray(input_var, np.float64)
    Wf = np.asarray(W, np.float64)
    bf = np.asarray(b, np.float64)
    tr = np.asarray(transitions, np.float64)
    mf = np.asarray(mask, np.float64)
    Bn, Sn, Dn = x.shape
    feats = (x.reshape(-1, Dn) @ Wf.T + bf).reshape(Bn, Sn, -1)
    fv = np.full((Bn, T), NEG)
    fv[:, START] = 0.0
    for t in range(Sn):
        tv = fv[:, None, :] + tr[None] + feats[:, t][:, :, None]
        m = tv.max(axis=2)
        new = m + np.log(np.exp(tv - m[:, :, None]).sum(axis=2))
        fv = new * mf[:, t : t + 1] + fv * (1 - mf[:, t : t + 1])
    fin = fv + tr[STOP][None]
    mm = fin.max(axis=1)
    alpha = mm + np.log(np.exp(fin - mm[:, None]).sum(axis=1))
    score0 = tr[tags[:, 0], START]
    emit = np.take_along_axis(feats[:, :-1], tags[:, :-1, None], axis=2)[..., 0]
    emit_sum = (emit * mf[:, :-1]).sum(axis=1)
    trs = tr[tags[:, 1:], tags[:, :-1]]
    trans_sum = (trs * mf[:, 1:]).sum(axis=1)
    last_idx = np.asarray(mask).sum(axis=1).astype(np.int64) - 1
    last_tags = np.take_along_axis(tags, last_idx[:, None], axis=1)[:, 0]
    last_emit = np.take_along_axis(feats[:, -1], last_tags[:, None], axis=1)[:, 0]
    gold = score0 + emit_sum + trans_sum + tr[STOP, last_tags] + last_emit * mf[:, -1]
    return np.float32((alpha - gold).sum())


def _make_in_maps(input_var, W, b, tags):
    import ml_dtypes

    f8 = ml_dtypes.float8_e4m3
    # wt[p, dc, j] = 32*W[j, dc*128 + p]
    w32 = WSCALE * np.asarray(W[:NT], np.float32)
    wt = np.ascontiguousarray(
        w32.T.reshape(4, 128, NT).transpose(1, 0, 2)
    ).astype(f8)

    x8 = input_var.reshape(B * S, D).astype(f8)   # one big cast
    in_maps = []
    for c in range(N_CORES):
        xc = x8[c * R : (c + 1) * R]              # [4096, 512]
        # xt[k, p, dc, s] = x[k*512+s, dc*128 + p]
        xt = np.ascontiguousarray(
            xc.T.reshape(4, 128, N_SLAB, HALF).transpose(2, 1, 0, 3)
        )
        in_maps.append({"xt": xt, "wt": wt})
    return in_maps


def kernel(input_var, W, b, transitions, tags, mask):
    from concourse.bass_utils import run_bass_kernel_spmd

    input_var = np.asarray(input_var)
    W = np.asarray(W)
    b = np.asarray(b)
    transitions = np.asarray(transitions)
    tags = np.asarray(tags)
    mask = np.asarray(mask)

    if not _fast_path_ok(transitions, tags, mask):
        return _reference_numpy(input_var, W, b, transitions, tags, mask)

    nc = _get_nc()
    in_maps = _make_in_maps(input_var, W, b, tags)
    res = run_bass_kernel_spmd(nc, in_maps, list(range(N_CORES)))

    # out_e group (bank,g) holds half [6,7,0,1 | 2,3,4][...]; h5 is out_c
    Fb = np.stack([np.asarray(res.results[c]["out_e"]) for c in range(N_CORES)])
    Fb = Fb.reshape(N_CORES, 2, 4, 32, HALF)[:, :, :, :NT]   # drop junk rows
    Fc = np.stack([np.asarray(res.results[c]["out_c"]) for c in range(N_CORES)])
    Fg = Fb.reshape(N_CORES, 8, NT, HALF)
    F = np.empty_like(Fg)
    for slot, h in enumerate((6, 7, 0, 1, 2, 3, 4)):
        F[:, h] = Fg[:, slot]
    F[:, 5] = Fc
    f = F.astype(np.float64) / WSCALE + np.asarray(b, np.float64)[:NT][None, None, :, None]
    f = f.reshape(B, NT, S)                        # [b, j, t]
    m = f.max(axis=1)
    lse = m + np.log(np.exp(f - m[:, None, :]).sum(axis=1))   # [B, S]
    gold = np.take_along_axis(f, tags[:, None, :].astype(np.int64), axis=1)[:, 0]
    return np.float32((lse - gold).sum())


# revision 33
# speedup vs baseline: 1.1546x; 1.0252x over previous
"""Trainium2 Bass kernel for the DecoderCRF loss (B=64, S=512, D=512, T=12).

Math
----
reference loss = sum_b [ logZ_b - gold_b ] with feats = x @ W.T + b.

For the transitions matrix this problem ships (row START == -1e4, col
STOP == -1e4, everything else 0) and an all-ones mask, the forward
recursion collapses exactly (verified vs a float64 port of the reference):

    logZ_b  = sum_t log( sum_{j=0..9} exp(feats[b,t,j]) )
    gold_b  = sum_t feats[b,t,tags[b,t]]

Layout strategy (final; 38.1us baseline -> ~21.4-22.1us measured)
------------------------------------------------------------
v2 was tensor-engine bound (59 matmuls = 27us busy incl the HAM cold
clock, plus 6us ScalarE exp + 5us DVE on 10-partition tiles).  Now:

* x ships as fp8(e4m3), W pre-scaled by 32 in fp8 -> DMA halves to
  2MB/core.  Host-simulated pipeline rel err: 3.6e-05 (gate 2e-2).
* Plain fp8 matmuls (DoubleRow streams 2N rhs entries - no cycle win -
  and rejects tile_position packing in walrus): 8 halves x 4 d-chunks
  of [128, 512].  PE col-tiling (tile_position=(0,32g)) packs up to 4
  halves' [10,512] outputs into one PSUM bank at offsets 0/32/64/96.
* DMA routing by measured queue behavior: the SWDGE (gpsimd) path
  streams ~300GB/s but its first transfer lands ~8us after emission,
  so it carries h0-h5; the two HWDGE queues (sync/scalar - start
  immediately, ~100GB/s each) carry h6/h7, and the PE consumes THOSE
  first so it never waits for the SWDGE pipe to wake.  (Giving HWDGE
  more than 2 slabs crowds the shared SDMA engines and delays the
  SWDGE wake - measured net loss.)
* A burst of warmup matmuls on memset-zero tiles runs while the x DMA
  streams, burning the HAM cold-clock window (PE ramps 1.2->2.4GHz
  after ~3.4us of sustained activity) so real matmuls run warm
  (215ns issue cadence per [128,512] when the package isn't power-
  throttled by the other 7 cores running the same kernel).
* Evacuation: banks A(h6,h7,h0,h1)/B(h2-h4) each get one wide DVE
  tensor_copy [128,512] f32->bf16 + one 128KB sync-queue DMA out,
  both completing during later matmuls; the last-consumed h5 sits in
  its own PSUM bank so only a [10,512] cast + 10KB out trail the
  final matmul.  Junk partitions between the 10-row groups ship and
  are ignored on host.
* No on-device exp/reductions: feats ship out and the O(B*S*T) finish
  (exp/log/sum/gather in f64) runs on host, like v2's log/bias finish.
* Remaining time is mostly fixed cost: ~4us NEFF startup (engine
  program loads + barriers), ~2.4us teardown, ~8us SWDGE pipe latency
  (warmup-overlapped), stream ~7us, tail ~2.5us.

Non-conforming inputs (different transitions pattern / mask / tag range)
fall back to a faithful numpy port of the reference.
"""

from contextlib import ExitStack

import numpy as np

N_CORES = 8
B, S, D = 64, 512, 512
T = 12
NT = 10          # tags that can actually appear / participate in the LSE
START, STOP = 10, 11
NEG = -10000.0
BS = B // N_CORES          # batch elements per core
R = BS * S                 # s-rows per core (4096)
N_HALF = 8                 # 512-col halves per core (one batch element each)
HALF = R // N_HALF         # 512
N_SLAB = 8                 # x DMA slabs per core (one half each)
N_WARM = 7                 # HAM warmup matmuls issued before real data lands
WSCALE = 32.0              # W is shipped as 32*W in fp8; host divides out

_NC_CACHE = None


def _build_nc():
    import concourse.bacc as bacc
    import concourse.mybir as mybir
    import concourse.tile as tile

    f32 = mybir.dt.float32
    bf16 = mybir.dt.bfloat16
    f8 = mybir.dt.float8e4
    nc = bacc.Bacc("TRN2", target_bir_lowering=False, num_swdge_queues=1)

    # slab k holds half k: [partition p, dc, s] with d = dc*128 + p,
    # global row = 512*k + s.  Per-partition data is one contiguous 2KB run.
    xt_d = nc.dram_tensor("xt", [N_SLAB, 128, 4, HALF], f8, kind="ExternalInput")
    wt_d = nc.dram_tensor("wt", [128, 4, NT], f8, kind="ExternalInput")
    out_d = nc.dram_tensor("out_e", [2, 128, HALF], bf16, kind="ExternalOutput")
    outc_d = nc.dram_tensor("out_c", [NT, HALF], bf16, kind="ExternalOutput")

    with tile.TileContext(nc) as tc, ExitStack() as ctx:
        consts = ctx.enter_context(tc.tile_pool(name="consts", bufs=1))
        xp = ctx.enter_context(tc.tile_pool(name="xp", bufs=N_SLAB))
        ep = ctx.enter_context(tc.tile_pool(name="ep", bufs=2))
        pw = ctx.enter_context(tc.tile_pool(name="pw", bufs=1, space="PSUM"))
        pp = ctx.enter_context(tc.tile_pool(name="pp", bufs=2, space="PSUM"))

        # tiny SWDGE kick: absorbs any one-time SWDGE/SDMA startup latency
        kick_sb = consts.tile([1, 64], f8, tag="kick")
        nc.gpsimd.dma_start(out=kick_sb, in_=xt_d[0, 0, 0, 0:64])

        # h0-h5 stream on the fast SWDGE path in consumption order; the two
        # HWDGE queues (slower but independent) each carry one of the LAST-
        # emitted halves, which the PE consumes FIRST (they arrive ~7us
        # while the SWDGE pipe wakes at ~11.5us).  A third HWDGE slab
        # delays the SWDGE wake - measured net loss.
        xt_tiles = []
        for k in range(N_SLAB):
            xt_sb = xp.tile([128, 4, HALF], f8, tag="xt")
            if k == 6:
                nc.sync.dma_start(out=xt_sb, in_=xt_d[k])
            elif k == 7:
                nc.scalar.dma_start(out=xt_sb, in_=xt_d[k])
            else:
                nc.gpsimd.dma_start(out=xt_sb, in_=xt_d[k])
            xt_tiles.append(xt_sb)

        wt_sb = consts.tile([128, 4, NT], f8)
        nc.sync.dma_start(out=wt_sb, in_=wt_d[:, :, :])

        # HAM warmup: zero matmuls with no DMA dependency keep the PE busy
        # through its ~3.4us cold-clock window while x streams in.
        wz = consts.tile([128, NT], f8, tag="wz")
        nc.vector.memset(wz, 0.0)
        xz = consts.tile([128, HALF], f8, tag="xz")
        nc.vector.memset(xz, 0.0)
        ps_w = pw.tile([NT, HALF], f32, tag="psw")
        for _ in range(N_WARM):
            nc.tensor.matmul(ps_w, lhsT=wz, rhs=xz, start=True, stop=True)


        # PE consumes h6/h7 FIRST, then chases the SWDGE stream h0..h5.
        # The last-consumed h5 sits alone in bank C so only a [10,512]
        # cast + 10KB out trail the final matmul.
        BANK_HALVES = ((6, 7, 0, 1), (2, 3, 4))
        for bank, halves in enumerate(BANK_HALVES):
            ps = pp.tile([128, HALF], f32, tag="ps")
            for g, h in enumerate(halves):
                for dc in range(4):   # four 128-deep d-chunks
                    nc.tensor.matmul(
                        ps[32 * g : 32 * g + NT, :],
                        lhsT=wt_sb[:, dc],
                        rhs=xt_tiles[h][:, dc],
                        start=(dc == 0),
                        stop=(dc == 3),
                        tile_position=(0, 32 * g),
                    )
                if h == 7:
                    # bridge the ~2.4us SWDGE-wake stall so the PE clock
                    # stays hot into the h0..h5 stream
                    for _ in range(3):
                        nc.tensor.matmul(ps_w, lhsT=wz, rhs=xz,
                                         start=True, stop=True)
            e_sb = ep.tile([128, HALF], bf16, tag="e")
            nc.vector.tensor_copy(out=e_sb, in_=ps)
            nc.sync.dma_start(out=out_d[bank], in_=e_sb)

        ps_c = pp.tile([NT, HALF], f32, tag="psc")
        for dc in range(4):
            nc.tensor.matmul(
                ps_c,
                lhsT=wt_sb[:, dc],
                rhs=xt_tiles[5][:, dc],
                start=(dc == 0),
                stop=(dc == 3),
            )
        ec_sb = ep.tile([NT, HALF], bf16, tag="ec")
        nc.vector.tensor_copy(out=ec_sb, in_=ps_c)
        # scalar HWDGE queue is idle by now: the tail 10KB out emits in
        # parallel with bank B's 128KB out on sync instead of behind it
        nc.scalar.dma_start(out=outc_d[:, :], in_=ec_sb)

    nc.compile()
    return nc


def _get_nc():
    global _NC_CACHE
    if _NC_CACHE is None:
        _NC_CACHE = _build_nc()
    return _NC_CACHE


def _fast_path_ok(transitions, tags, mask):
    if transitions.shape != (T, T) or tags.min() < 0 or tags.max() >= NT:
        return False
    if not np.all(mask == 1):
        return False
    t2 = np.asarray(transitions, np.float64).copy()
    if not (np.all(t2[START, :] == NEG) and np.all(t2[:, STOP] == NEG)):
        return False
    t2[START, :] = 0.0
    t2[:, STOP] = 0.0
    return bool(np.all(t2 == 0.0))


def _reference_numpy(input_var, W, b, transitions, tags, mask):
    """Faithful float64 port of the reference (fallback only)."""
    x = np.asarray(input_var, np.float64)
    Wf = np.asarray(W, np.float64)
    bf = np.asarray(b, np.float64)
    tr = np.asarray(transitions, np.float64)
    mf = np.asarray(mask, np.float64)
    Bn, Sn, Dn = x.shape
    feats = (x.reshape(-1, Dn) @ Wf.T + bf).reshape(Bn, Sn, -1)
    fv = np.full((Bn, T), NEG)
    fv[:, START] = 0.0
    for t in range(Sn):
        tv = fv[:, None, :] + tr[None] + feats[:, t][:, :, None]
        m = tv.max(axis=2)
        new = m + np.log(np.exp(tv - m[:, :, None]).sum(axis=2))
        fv = new * mf[:, t : t + 1] + fv * (1 - mf[:, t : t + 1])
    fin = fv + tr[STOP][None]
    mm = fin.max(axis=1)
    alpha = mm + np.log(np.exp(fin - mm[:, None]).sum(axis=1))
    score0 = tr[tags[:, 0], START]
    emit = np.take_along_axis(feats[:, :-1], tags[:, :-1, None], axis=2)[..., 0]
    emit_sum = (emit * mf[:, :-1]).sum(axis=1)
    trs = tr[tags[:, 1:], tags[:, :-1]]
    trans_sum = (trs * mf[:, 1:]).sum(axis=1)
    last_idx = np.asarray(mask).sum(axis=1).astype(np.int64) - 1
    last_tags = np.take_along_axis(tags, last_idx[:, None], axis=1)[:, 0]
    last_emit = np.take_along_axis(feats[:, -1], last_tags[:, None], axis=1)[:, 0]
    gold = score0 + emit_sum + trans_sum + tr[STOP, last_tags] + last_emit * mf[:, -1]
    return np.float32((alpha - gold).sum())


def _make_in_maps(input_var, W, b, tags):
    import ml_dtypes

    f8 = ml_dtypes.float8_e4m3
    # wt[p, dc, j] = 32*W[j, dc*128 + p]
    w32 = WSCALE * np.asarray(W[:NT], np.float32)
    wt = np.ascontiguousarray(
        w32.T.reshape(4, 128, NT).transpose(1, 0, 2)
    ).astype(f8)

    x8 = input_var.reshape(B * S, D).astype(f8)   # one big cast
    in_maps = []
    for c in range(N_CORES):
        xc = x8[c * R : (c + 1) * R]              # [4096, 512]
        # xt[k, p, dc, s] = x[k*512+s, dc*128 + p]
        xt = np.ascontiguousarray(
            xc.T.reshape(4, 128, N_SLAB, HALF).transpose(2, 1, 0, 3)
        )
        in_maps.append({"xt": xt, "wt": wt})
    return in_maps


def kernel(input_var, W, b, transitions, tags, mask):
    from concourse.bass_utils import run_bass_kernel_spmd

    input_var = np.asarray(input_var)
    W = np.asarray(W)
    b = np.asarray(b)
    transitions = np.asarray(transitions)
    tags = np.asarray(tags)
    mask = np.asarray(mask)

    if not _fast_path_ok(transitions, tags, mask):
        return _reference_numpy(input_var, W, b, transitions, tags, mask)

    nc = _get_nc()
    in_maps = _make_in_maps(input_var, W, b, tags)
    res = run_bass_kernel_spmd(nc, in_maps, list(range(N_CORES)))

    # out_e group (bank,g) holds half [6,7,0,1 | 2,3,4][...]; h5 is out_c
    Fb = np.stack([np.asarray(res.results[c]["out_e"]) for c in range(N_CORES)])
    Fb = Fb.reshape(N_CORES, 2, 4, 32, HALF)[:, :, :, :NT]   # drop junk rows
    Fc = np.stack([np.asarray(res.results[c]["out_c"]) for c in range(N_CORES)])
    Fg = Fb.reshape(N_CORES, 8, NT, HALF)
    F = np.empty_like(Fg)
    for slot, h in enumerate((6, 7, 0, 1, 2, 3, 4)):
        F[:, h] = Fg[:, slot]
    F[:, 5] = Fc
    f = F.astype(np.float64) / WSCALE + np.asarray(b, np.float64)[:NT][None, None, :, None]
    f = f.reshape(B, NT, S)                        # [b, j, t]
    m = f.max(axis=1)
    lse = m + np.log(np.exp(f - m[:, None, :]).sum(axis=1))   # [B, S]
    gold = np.take_along_axis(f, tags[:, None, :].astype(np.int64), axis=1)[:, 0]
    return np.float32((lse - gold).sum())
